# revision 4
# baseline (speedup 1.0000x reference)
"""Compensated sparse linear: out = x @ (W + delta_B)^T + b on 8 NeuronCores.

Both terms contract x against [out, in] matrices, so the module is one GEMM
with V = W + delta_B, plus bias. Inputs are cast to bf16 on host (~0.3% rel
err, tolerance is 2e-2): PE runs bf16 at the same 1 row/cycle as fp32r but
FWL halves the weight-load cost, and DMA traffic halves.

Sharding (hardcoded for x:[4,2048,4096], W/delta_B:[4096,4096], b:[4096]):
  2 token shards x 4 out-feature shards -> 8 cores; core = r*4 + c.
  Per core: x2d shard [4096, 4096] tokens x K, V shard [1024, 4096].

Device kernel (per core), feature-partition PSUM:
  stationary = V^T tile [128K, 128 feat] (resident, 64KB/partition bf16)
  moving     = x^T tile [128K, 512 tok]  (streamed per t-tile, 32KB/part)
  psum       = [128 feat, 512 tok], one bank
Per (t-tile, f-window): 32 matmuls over kt, then ONE ScalarE activation
does bias-add (bias is per-partition in this orientation) + PSUM->SBUF
copy; DMA writes the transposed output od[N_C, T_C] (host untransposes).
"""

import numpy as np
import ml_dtypes

import concourse.tile as tile
from concourse import bacc, mybir
from concourse.bass_utils import run_bass_kernel_spmd

P = 128
B, S, D_IN, D_OUT = 4, 2048, 4096, 4096
T = B * S
TR, NCOLS = 2, 4            # token shards x feature shards
T_C, N_C = T // TR, D_OUT // NCOLS
K = D_IN
KT = K // P                 # 32 k-tiles
TOK = 512                   # moving free dim (one PSUM bank fp32)
TT = T_C // TOK             # 8 t-tiles
FW = N_C // P               # 8 feature windows
BF = mybir.dt.bfloat16
NPBF = ml_dtypes.bfloat16


def build_nc(reps=1, bench_mode=False):
    """bench_mode: big tensors become Internal DRAM scratch (no host upload /
    download per dispatch) with tiny dummy io, so chained-dispatch timing
    measures pure device time. Instruction stream is identical."""
    nc = bacc.Bacc("TRN2", target_bir_lowering=False, debug=False, num_devices=8)
    big = "Internal" if bench_mode else "ExternalInput"
    bigo = "Internal" if bench_mode else "ExternalOutput"
    xd = nc.dram_tensor("xt", [TT, P, KT, TOK], BF, kind=big).ap()
    vd = nc.dram_tensor("vt", [FW, P, KT, P], BF, kind=big).ap()
    bd = nc.dram_tensor("bias", [P, FW], mybir.dt.float32, kind=big).ap()
    od = nc.dram_tensor("out", [N_C, T_C], mybir.dt.float32, kind=bigo).ap()
    if bench_mode:
        tin = nc.dram_tensor("tin", [P, P], mybir.dt.float32, kind="ExternalInput").ap()
        tout = nc.dram_tensor("tout", [P, P], mybir.dt.float32, kind="ExternalOutput").ap()

    ident = mybir.ActivationFunctionType.Identity

    with tile.TileContext(nc) as tc:
        with (
            tc.tile_pool(name="v", bufs=1) as v_pool,
            tc.tile_pool(name="bias", bufs=1) as b_pool,
            tc.tile_pool(name="x", bufs=3) as x_pool,
            tc.tile_pool(name="outp", bufs=6) as out_pool,
            tc.tile_pool(name="psum", bufs=8, space="PSUM") as psum_pool,
        ):
            for rep in range(reps):
                bias_s = b_pool.tile([P, FW], mybir.dt.float32)
                v_s = v_pool.tile([P, FW, KT, P], BF, name="v")

                # critical path: first x tile, then V windows in consumption order
                x_tiles = {}
                x_s = x_pool.tile([P, KT, TOK], BF)
                nc.sync.dma_start(x_s[:], xd[0])
                x_tiles[0] = x_s
                nc.sync.dma_start(bias_s[:], bd[:])
                for fw in range(FW):
                    nc.sync.dma_start(v_s[:, fw, :, :], vd[fw])
                x_s = x_pool.tile([P, KT, TOK], BF)
                nc.sync.dma_start(x_s[:], xd[1])
                x_tiles[1] = x_s

                for tt in range(TT):
                    if tt + 2 < TT:
                        x_s = x_pool.tile([P, KT, TOK], BF)
                        nc.sync.dma_start(x_s[:], xd[tt + 2])
                        x_tiles[tt + 2] = x_s
                    xt_s = x_tiles.pop(tt)
                    for fw in range(FW):
                        ps = psum_pool.tile([P, TOK], mybir.dt.float32)
                        for kt in range(KT):
                            nc.tensor.matmul(
                                ps[:], v_s[:, fw, kt, :], xt_s[:, kt, :],
                                start=(kt == 0), stop=(kt == KT - 1),
                            )
                        o = out_pool.tile([P, TOK], mybir.dt.float32)
                        nc.scalar.activation(
                            o[:], ps[:], ident, bias=bias_s[:, fw:fw + 1], scale=1.0
                        )
                        nc.sync.dma_start(
                            od[fw * P:(fw + 1) * P, tt * TOK:(tt + 1) * TOK], o[:]
                        )
            if bench_mode:
                t_s = b_pool.tile([P, P], mybir.dt.float32, name="tin")
                nc.sync.dma_start(t_s[:], tin[:])
                nc.sync.dma_start(tout[:], t_s[:])
    nc.compile()
    return nc


def shard_layout():
    return [(r, c) for r in range(TR) for c in range(NCOLS)]


def prepare_in_maps(x, W, b, delta_B):
    x2d = np.asarray(x, np.float32).reshape(T, D_IN)
    V = np.asarray(W, np.float32) + np.asarray(delta_B, np.float32)
    b = np.asarray(b, np.float32)

    in_maps = []
    for r, c in shard_layout():
        xs = x2d[r * T_C:(r + 1) * T_C]
        xt = np.ascontiguousarray(
            xs.reshape(TT, TOK, KT, P).transpose(0, 3, 2, 1).astype(NPBF)
        )
        Vc = V[c * N_C:(c + 1) * N_C]
        vt = np.ascontiguousarray(
            Vc.reshape(FW, P, KT, P).transpose(0, 3, 2, 1).astype(NPBF)
        )
        bias = np.ascontiguousarray(b[c * N_C:(c + 1) * N_C].reshape(FW, P).T)
        in_maps.append({"xt": xt, "vt": vt, "bias": bias})
    return in_maps


def assemble_output(results):
    out = np.empty((T, D_OUT), np.float32)
    for i, (r, c) in enumerate(shard_layout()):
        out[r * T_C:(r + 1) * T_C, c * N_C:(c + 1) * N_C] = results[i]["out"].T
    return out.reshape(B, S, D_OUT)


def kernel(x, W, b, delta_B):
    nc = build_nc()
    in_maps = prepare_in_maps(x, W, b, delta_B)
    res = run_bass_kernel_spmd(nc, in_maps, list(range(8)))
    return assemble_output(res.results)


# revision 5
# speedup vs baseline: 1.4434x; 1.4434x over previous
"""Compensated sparse linear: out = x @ (W + delta_B)^T + b on 8 NeuronCores.

Both terms contract x against [out, in] matrices, so the module is one GEMM
with V = W + delta_B, plus bias. Inputs are cast to bf16 on host (~0.3% rel
err, tolerance 2e-2): PE runs bf16 at 1 row/cycle like fp32r but FWL halves
weight-load cost and DMA traffic halves.

Sharding (hardcoded for x:[4,2048,4096], W/delta_B:[4096,4096], b:[4096]):
  2 token shards x 4 out-feature shards -> 8 cores; core = r*4 + c.
  Per core: x2d shard [4096, 4096] tokens x K, V shard [1024, 4096].

Device kernel (per core), feature-partition PSUM:
  stationary = V^T tile [128K, 128 feat] (resident, 64KB/partition bf16)
  moving     = x^T tile [128K, 512 tok]  (streamed per t-tile, 32KB/part)
  psum       = [128 feat, 512 tok], one bank
Per (t-tile, f-window): 32 matmuls over kt, then ONE ScalarE activation does
bias-add (per-partition in this orientation) + PSUM->SBUF copy; DMA writes
od[tt, N_C, TOK] (host reassembles/transposes).

The 64 groups run inside a hardware For_i loop over t-tile PAIRS (body =
16 groups, x double-buffered A/B, dram addresses register-offset by the
loop var): a fully unrolled 2048-matmul stream bottlenecks the PE
sequencer on instruction fetch (~330 ns/mm vs ~190 looped).
"""

import numpy as np
import ml_dtypes

import concourse.tile as tile
from concourse import bacc, mybir
from concourse.bass_utils import run_bass_kernel_spmd

P = 128
B, S, D_IN, D_OUT = 4, 2048, 4096, 4096
T = B * S
TR, NCOLS = 2, 4            # token shards x feature shards
T_C, N_C = T // TR, D_OUT // NCOLS
K = D_IN
KT = K // P                 # 32 k-tiles
TOK = 512                   # moving free dim (one PSUM bank fp32)
TT = T_C // TOK             # 8 t-tiles
FW = N_C // P               # 8 feature windows
BF = mybir.dt.bfloat16
NPBF = ml_dtypes.bfloat16


def build_nc(reps=1, bench_mode=False):
    """bench_mode: big tensors become Internal DRAM scratch (no host upload /
    download per dispatch) with tiny dummy io, so chained-dispatch timing
    measures pure device time. Instruction stream is identical."""
    nc = bacc.Bacc("TRN2", target_bir_lowering=False, debug=False, num_devices=8)
    big = "Internal" if bench_mode else "ExternalInput"
    bigo = "Internal" if bench_mode else "ExternalOutput"
    # xt padded to TT+1 tiles: the loop prefetches xd[tt0+2] up to index TT.
    xd = nc.dram_tensor("xt", [TT + 1, P, KT, TOK], BF, kind=big).ap()
    vd = nc.dram_tensor("vt", [FW, P, KT, P], BF, kind=big).ap()
    bd = nc.dram_tensor("bias", [P, FW], mybir.dt.float32, kind=big).ap()
    od = nc.dram_tensor("out", [TT, N_C, TOK], mybir.dt.float32, kind=bigo).ap()
    if bench_mode:
        tin = nc.dram_tensor("tin", [P, P], mybir.dt.float32, kind="ExternalInput").ap()
        tout = nc.dram_tensor("tout", [P, P], mybir.dt.float32, kind="ExternalOutput").ap()

    ident = mybir.ActivationFunctionType.Identity

    with tile.TileContext(nc) as tc:
        with (
            tc.tile_pool(name="v", bufs=1) as v_pool,
            tc.tile_pool(name="bias", bufs=1) as b_pool,
            tc.tile_pool(name="x", bufs=2) as x_pool,
            tc.tile_pool(name="outp", bufs=6) as out_pool,
            tc.tile_pool(name="psum", bufs=8, space="PSUM") as psum_pool,
        ):
            for rep in range(reps):
                bias_s = b_pool.tile([P, FW], mybir.dt.float32)
                v_s = v_pool.tile([P, FW, KT, P], BF, name="v")
                x_a = x_pool.tile([P, KT, TOK], BF, name="xa")
                x_b = x_pool.tile([P, KT, TOK], BF, name="xb")

                nc.sync.dma_start(x_a[:], xd[0])
                nc.sync.dma_start(bias_s[:], bd[:])
                for fw in range(FW):
                    nc.sync.dma_start(v_s[:, fw, :, :], vd[fw])

                def groups(xt_s, od_tt):
                    for fw in range(FW):
                        ps = psum_pool.tile([P, TOK], mybir.dt.float32)
                        for kt in range(KT):
                            nc.tensor.matmul(
                                ps[:], v_s[:, fw, kt, :], xt_s[:, kt, :],
                                start=(kt == 0), stop=(kt == KT - 1),
                            )
                        o = out_pool.tile([P, TOK], mybir.dt.float32)
                        nc.scalar.activation(
                            o[:], ps[:], ident, bias=bias_s[:, fw:fw + 1], scale=1.0
                        )
                        nc.sync.dma_start(od_tt[fw * P:(fw + 1) * P, :], o[:])

                with tc.For_i(0, TT, 2) as tt0:
                    nc.sync.dma_start(x_b[:], xd[tt0 + 1])
                    groups(x_a, od[tt0])
                    nc.sync.dma_start(x_a[:], xd[tt0 + 2])
                    groups(x_b, od[tt0 + 1])

            if bench_mode:
                t_s = b_pool.tile([P, P], mybir.dt.float32, name="tin")
                nc.sync.dma_start(t_s[:], tin[:])
                nc.sync.dma_start(tout[:], t_s[:])
    nc.compile()
    return nc


def shard_layout():
    return [(r, c) for r in range(TR) for c in range(NCOLS)]


def prepare_in_maps(x, W, b, delta_B):
    x2d = np.asarray(x, np.float32).reshape(T, D_IN)
    V = np.asarray(W, np.float32) + np.asarray(delta_B, np.float32)
    b = np.asarray(b, np.float32)

    in_maps = []
    for r, c in shard_layout():
        xs = x2d[r * T_C:(r + 1) * T_C]
        xt = np.zeros((TT + 1, P, KT, TOK), NPBF)
        xt[:TT] = xs.reshape(TT, TOK, KT, P).transpose(0, 3, 2, 1).astype(NPBF)
        Vc = V[c * N_C:(c + 1) * N_C]
        vt = np.ascontiguousarray(
            Vc.reshape(FW, P, KT, P).transpose(0, 3, 2, 1).astype(NPBF)
        )
        bias = np.ascontiguousarray(b[c * N_C:(c + 1) * N_C].reshape(FW, P).T)
        in_maps.append({"xt": xt, "vt": vt, "bias": bias})
    return in_maps


def assemble_output(results):
    out = np.empty((T, D_OUT), np.float32)
    for i, (r, c) in enumerate(shard_layout()):
        blk = results[i]["out"].transpose(1, 0, 2).reshape(N_C, T_C)
        out[r * T_C:(r + 1) * T_C, c * N_C:(c + 1) * N_C] = blk.T
    return out.reshape(B, S, D_OUT)


def kernel(x, W, b, delta_B):
    nc = build_nc()
    in_maps = prepare_in_maps(x, W, b, delta_B)
    res = run_bass_kernel_spmd(nc, in_maps, list(range(8)))
    return assemble_output(res.results)
